# revision 34
# baseline (speedup 1.0000x reference)
"""Channel-attention transformer block on 8 Trainium2 NeuronCores.

Reference semantics (b=8, l=4096, c=512, h=8 heads carved from the
*sequence* axis, head_pos = l % 8):
    qkv = x @ w_qkv.T ; split q,k,v per head  (each (lh=512, c=512))
    attn = softmax((q.T @ k) / 8, axis=-1)    # (c, c) channel attention
    y.T  = attn @ v.T                         # (c, lh)
    out  = y @ w_out.T + b_out

Sharding: data-parallel over batch — core i handles batch i.

Per-core layout trick: the l axis is permuted on the host so each head's
512 rows are contiguous (row h*512+i <- original row i*8+h), and x is
shipped transposed (c, l). Then per head:
  - Q,K in natural (l, c) layout and V^T in (c, l) layout all come
    straight out of matmuls against xT (no on-device transposes),
  - scores are computed *transposed* (S^T = K^T Q via lhsT=K, rhs=Q) so
    softmax's sum over the attended axis lands on the partition dim,
    where it is computed by a matmul against ones columns glued onto
    V^T (columns 0-1 of the AV rhs) — again no transposes,
  - normalization (multiply by 1/denominator, a per-partition scalar)
    is fused into the PSUM->SBUF evacuation of the AV result,
  - the out-projection consumes y^T (c on partitions) directly as lhsT.
The host un-permutes rows of the returned (4096, 512) per-core output.
"""

import numpy as np

import concourse.bass as bass
import concourse.mybir as mybir
import concourse.tile as tile
from concourse.bass_utils import run_bass_kernel_spmd

B = 8
L = 4096
C = 512
HEADS = 8
LH = L // HEADS  # 512
SCALE = 64 ** -0.5  # DIM_HEAD ** -0.5 from the reference
N_CORES = 8
P = 128
KC = C // P  # 4 contraction chunks of 128
F32 = mybir.dt.float32

# Matmul operand dtype: float32r is the PE's fast-fp32 mode (1 col/cycle
# for free dim >= 256, vs 4 cycles for strict fp32, ~TF32 precision). The
# BIR verifier requires every producer of a matmul input to emit float32r,
# so all tiles on the matmul path carry this dtype end to end.
MM_DTYPE = mybir.dt.float32r
PD = MM_DTYPE  # dtype of every tile that feeds a matmul


def _split_wide_waits(nc, max_waits=1):
    """This container's walrus build rejects instructions carrying more than
    ~1 sync wait ("Too many sync wait commands", e.g. in the S3_LW lowering
    of a fused matmul). Hoist surplus waits onto same-engine nops inserted
    immediately before the offending instruction — the engine stalls at the
    same point in its stream, so scheduling semantics are unchanged."""
    for f in nc.m.functions:
        for bb in f.blocks:
            snapshot = list(bb.instructions)
            if not any(
                inst.sync_info and inst.sync_info.on_wait
                and len(inst.sync_info.on_wait) > max_waits
                for inst in snapshot
            ):
                continue
            new = []
            for inst in snapshot:
                si = inst.sync_info
                waits = list(si.on_wait) if si and si.on_wait else []
                if len(waits) > max_waits:
                    for w in waits[:-max_waits]:
                        nop = nc.engines[inst.engine].nop(nofuse=True).ins
                        cur = nc.cur_bb.bb.instructions
                        assert cur[-1] is nop
                        cur.pop()  # re-homed below, right before `inst`
                        nop.sync_info = mybir.SyncInfo(on_wait=[w], on_update=[])
                        new.append(nop)
                    inst.sync_info = mybir.SyncInfo(
                        on_wait=waits[-max_waits:],
                        on_update=list(si.on_update) if si.on_update else [],
                    )
                new.append(inst)
            bb.instructions = new


def _emit(ctx, tc, xt, wqkv_t, wout_t, ones2, out):
    """Emit the per-core program. All DRAM APs:
    xt (C, L) fp32, wqkv_t (C, 3C) fp32 (q block pre-scaled), wout_t (C, C),
    out (L, C)."""
    nc = tc.nc
    EXP = mybir.ActivationFunctionType.Exp

    xt_r = xt.rearrange("(ko p) l -> p ko l", p=P)
    wqkv_r = wqkv_t.rearrange("(ko p) n -> p ko n", p=P)
    wout_r = wout_t.rearrange("(ko p) n -> p ko n", p=P)

    consts = ctx.enter_context(tc.tile_pool(name="consts", bufs=1))
    xt_pool = ctx.enter_context(tc.tile_pool(name="xt", bufs=3))
    q_pool = ctx.enter_context(tc.tile_pool(name="q", bufs=3))
    k_pool = ctx.enter_context(tc.tile_pool(name="k", bufs=3))
    vt_pool = ctx.enter_context(tc.tile_pool(name="vt", bufs=3))
    exp_pool = ctx.enter_context(tc.tile_pool(name="exp", bufs=3))
    y_pool = ctx.enter_context(tc.tile_pool(name="y", bufs=3))
    out_pool = ctx.enter_context(tc.tile_pool(name="out", bufs=8))
    recip_pool = ctx.enter_context(tc.tile_pool(name="recip", bufs=8))
    pp_mm = ctx.enter_context(tc.tile_pool(name="pp_mm", bufs=4, space="PSUM"))
    pp_y1 = ctx.enter_context(tc.tile_pool(name="pp_y1", bufs=2, space="PSUM"))
    pp_y2 = ctx.enter_context(tc.tile_pool(name="pp_y2", bufs=2, space="PSUM"))

    # Startup critical path: head 0's first matmul group needs its x block
    # strip plus the q-weights. Ship x blocks on the SWDGE rail (gpsimd) and
    # weights on the HWDGE rail (sync) so they transfer concurrently, with
    # head 0's block split into column strips to unblock the first group.
    wqkv = consts.tile([P, KC, 3 * C], PD)
    wout = consts.tile([P, KC, C], PD)
    xth0 = xt_pool.tile([P, KC, LH], PD, tag="xth")
    for m in range(KC):
        nc.gpsimd.dma_start(xth0[:, :, bass.ts(m, P)],
                            xt_r[:, :, bass.ds(m * P, P)])
    for j in range(3):
        for ko in range(KC):
            nc.sync.dma_start(wqkv[:, ko, bass.ts(j, C)],
                              wqkv_r[:, ko, bass.ts(j, C)])
    nc.sync.dma_start(wout[:], wout_r[:])

    for h in range(HEADS):
        if h == 0:
            xth = xth0
        else:
            xth = xt_pool.tile([P, KC, LH], PD, tag="xth")
            nc.gpsimd.dma_start(xth[:], xt_r[:, :, bass.ts(h, LH)])

        # ---- projections: Q,K natural (l, c); V^T (c, l) with ones col ----
        q = q_pool.tile([P, KC, C], PD)
        k = k_pool.tile([P, KC, C], PD)
        for m in range(KC):  # l' strips of 128
            for j, dst in ((0, q), (1, k)):
                pq = pp_mm.tile([P, C], F32, tag="mm")
                for ko in range(KC):
                    nc.tensor.matmul(
                        pq[:], xth[:, ko, bass.ts(m, P)],
                        wqkv[:, ko, bass.ts(j, C)],
                        start=(ko == 0), stop=(ko == KC - 1))
                nc.vector.tensor_copy(dst[:, m, :], pq[:])

        vt = vt_pool.tile([P, KC, LH + 2], PD)
        nc.sync.dma_start(vt[:, :, 0:2], ones2[:])
        for m in range(KC):  # c_v strips of 128
            pv = pp_mm.tile([P, LH], F32, tag="mm")
            for ko in range(KC):
                nc.tensor.matmul(
                    pv[:], wqkv[:, ko, bass.ds(2 * C + m * P, P)],
                    xth[:, ko, :],
                    start=(ko == 0), stop=(ko == KC - 1))
            nc.vector.tensor_copy(vt[:, m, 2:LH + 2], pv[:])

        # ---- scores transposed + exp:  S^T[d, c] = sum_l K[l,d] Q[l,c] ----
        ex = exp_pool.tile([P, KC, C], PD)
        for ds_ in range(KC):  # d strips of 128
            ps = pp_mm.tile([P, C], F32, tag="mm")
            for m in range(KC):  # contraction over l' chunks
                nc.tensor.matmul(
                    ps[:], k[:, m, bass.ts(ds_, P)],
                    q[:, m, :],
                    start=(m == 0), stop=(m == KC - 1))
            nc.scalar.activation(ex[:, ds_, :], ps[:], EXP)

        # ---- AV with fused denominator (rhs cols 0,1 are ones; two even
        # N-splits keep the fp32r matmul dst-pattern restriction happy) ----
        NY1 = 258  # 2 (denominator twice) + 256 v columns
        NY2 = 256
        y = y_pool.tile([P, KC, LH], PD)
        for cs in range(KC):  # c strips of 128
            py1 = pp_y1.tile([P, NY1], F32, tag="y1")
            py2 = pp_y2.tile([P, NY2], F32, tag="y2")
            for ko in range(KC):  # contraction over d chunks
                lhsT = ex[:, ko, bass.ts(cs, P)]
                nc.tensor.matmul(py1[:], lhsT, vt[:, ko, 0:NY1],
                                 start=(ko == 0), stop=(ko == KC - 1))
            for ko in range(KC):
                lhsT = ex[:, ko, bass.ts(cs, P)]
                nc.tensor.matmul(py2[:], lhsT, vt[:, ko, NY1:LH + 2],
                                 start=(ko == 0), stop=(ko == KC - 1))
            rc = recip_pool.tile([P, 1], F32)
            nc.vector.reciprocal(rc[:], py1[:, 0:1])
            nc.vector.tensor_scalar_mul(y[:, cs, 0:NY1 - 2], py1[:, 2:NY1], rc[:])
            nc.vector.tensor_scalar_mul(y[:, cs, NY1 - 2:LH], py2[:], rc[:])

        # ---- out projection: out[l, co] = sum_c y^T[c, l] woutT[c, co] ----
        for m in range(KC):  # l' strips of 128
            po = pp_mm.tile([P, C], F32, tag="mm")
            for ko in range(KC):
                nc.tensor.matmul(
                    po[:], y[:, ko, bass.ts(m, P)],
                    wout[:, ko, :],
                    start=(ko == 0), stop=(ko == KC - 1))
            ot = out_pool.tile([P, C], F32)
            nc.vector.tensor_copy(ot[:], po[:])
            nc.sync.dma_start(out[bass.ds(h * LH + m * P, P), :], ot[:])


def _build_program():
    nc = bass.Bass(trn_type="TRN2", target_bir_lowering=False, debug=False,
                   num_devices=N_CORES)
    xt = nc.dram_tensor("xt", [C, L], PD, kind="ExternalInput").ap()
    wqkv_t = nc.dram_tensor("wqkv_t", [C, 3 * C], PD, kind="ExternalInput").ap()
    wout_t = nc.dram_tensor("wout_t", [C, C], PD, kind="ExternalInput").ap()
    ones2 = nc.dram_tensor("ones2", [P, KC, 2], PD, kind="ExternalInput").ap()
    out = nc.dram_tensor("out", [L, C], F32, kind="ExternalOutput").ap()

    from contextlib import ExitStack
    with tile.TileContext(nc) as tc:
        with ExitStack() as ctx:
            _emit(ctx, tc, xt, wqkv_t, wout_t, ones2, out)
    _split_wide_waits(nc)
    return nc


def _host_inputs(x, w_qkv, w_out):
    """Per-core input maps. Permute l so head h owns rows [h*512, (h+1)*512)
    (original row i*8+h -> permuted row h*512+i), transpose to (c, l)."""
    wqkv_t = np.ascontiguousarray(w_qkv.T).astype(np.float32).copy()
    wqkv_t[:, 0:C] *= SCALE  # fold the attention scale into the Q weights
    wout_t = np.ascontiguousarray(w_out.T).astype(np.float32)
    ones2 = np.ones((P, KC, 2), dtype=np.float32)
    in_maps = []
    for b in range(B):
        xb = x[b]  # (L, C); row l = i*8 + h
        x_perm = xb.reshape(LH, HEADS, C).transpose(1, 0, 2).reshape(L, C)
        xt = np.ascontiguousarray(x_perm.T)  # (C, L)
        in_maps.append({"xt": xt, "wqkv_t": wqkv_t, "wout_t": wout_t,
                        "ones2": ones2})
    return in_maps


def _unpermute(out_perm):
    """(L, C) with rows grouped by head -> original row order i*8+h."""
    return out_perm.reshape(HEADS, LH, C).transpose(1, 0, 2).reshape(L, C)


def kernel(x, w_qkv, w_out, b_out, _run_kwargs=None):
    x = np.asarray(x, dtype=np.float32)
    w_qkv = np.asarray(w_qkv, dtype=np.float32)
    w_out = np.asarray(w_out, dtype=np.float32)
    b_out = np.asarray(b_out, dtype=np.float32)

    nc = _build_program()
    in_maps = _host_inputs(x, w_qkv, w_out)
    res = run_bass_kernel_spmd(nc, in_maps, list(range(N_CORES)),
                               **(_run_kwargs or {}))
    out = np.empty((B, L, C), dtype=np.float32)
    for b in range(B):
        out[b] = _unpermute(res.results[b]["out"])
    out += b_out
    if _run_kwargs:
        kernel.last_result = res
    return out
